# revision 12
# baseline (speedup 1.0000x reference)
"""Trainium2 Bass kernel for EnhancedSpikingAudioNet (4-layer LIF SNN).

Network (eval mode): for t in 0..99:
    s1,m1 = LIF(x_t @ W1.T + b1, m1)
    s2,m2 = LIF(s1 @ W2.T + b2, m2)
    s3,m3 = LIF(s2 @ W3.T + b3, m3)
    s4,m4 = LIF(s3 @ W4.T + b4, m4)
returns m4 (final step), shape [B=256, 10].

LIF (snnTorch Leaky, reset_mechanism='subtract', beta=.95, thr=1):
    reset = (m_prev > 1);  m = beta*m_prev + cur - reset;  s = (m > 1)
Note reset(t) == spike(t-1), so only s needs materializing.

Strategy: data-parallel over batch (32 per core, 8 cores).  Inside a
core, time is blocked (TB=10): all matmuls for a block are batched over
the block's 10 steps (moving free dim N=320); only the per-step LIF
update is sequential (3 DVE ops per layer per step, all on one engine
so the serial chain carries no cross-engine semaphores).  Layout:
features on partitions (128-chunks), (t, batch) on the free dim.  PSUM
is drained to SBUF by ScalarE with the layer bias fused in.  Startup:
x block 0 and w1 stream in per-k-chunk interleaved so the PE starts
~3us in (L1 is k-outer to chase the DMA); remaining weights follow in
consumption order.

Numerics: the spike cascade amplifies matmul noise (a plain f32r
matmul gives ~16% output error; even exact-fp32 summation-order noise
gives ~1.6%), so matmuls must be fp32-faithful.  Hardware probing
established: float32r = operands rounded RNE to 11 mantissa bits
(FP22 e10m11), then EXACT products with clean fp32 accumulation (a
pre-rounded-operand probe matches an exact model to 1e-7), at full PE
rate (1 cyc/row) for moving dims >= 256.  Therefore every fp32 tensor
is split host-side into two 11-bit planes (h = rne11(a), l = a-h; the
residual fits 12 significand bits, so a == h+l exactly and both planes
survive the hw operand rounding unchanged).  Weights use 2 planes;
spikes are 0/1 (f32r-exact, single copy); x uses 2 planes.  Layer 1
accumulates wh@xh + wh@xl + wl@xh (dropping wl@xl ~ 2^-24), layers 2-4
accumulate wh@s + wl@s.  Everything streams at 1 cycle/row and the
result is fp32-exact (hw-validated: bit-identical to the jax CPU
reference).
"""

import os
import sys

import numpy as np

for _p in ("/opt/trn_rl_repo",):
    if os.path.isdir(_p) and _p not in sys.path:
        sys.path.insert(0, _p)

import concourse.bass as bass
import concourse.mybir as mybir
import concourse.tile as tile
from concourse import bass_utils

F32 = mybir.dt.float32
F32R = mybir.dt.float32r
ALU = mybir.AluOpType
ACTF = mybir.ActivationFunctionType
PLANES = ("h", "l")  # 11-bit f32r planes


def _patch_tail_drain():
    """This container's walrus allows only ONE sync-wait on a Drain
    instruction; Tile's kernel-tail drain can carry several (one per DMA
    HW queue).  Spread the waits across consecutive drains instead."""
    from concourse.vector_clock import ScopedClock

    if getattr(tile.TileContext, "_tail_drain_patched", False):
        return

    def _drain_and_barrier(self, tick_clock, wait_clock):
        drain_inst = self.nc.sync.drain()
        wait_clock.add_sem_waits(
            drain_inst.ins, ScopedClock({None: tick_clock.global_clock})
        )
        si = drain_inst.ins.sync_info
        if si is not None and si.on_wait and len(si.on_wait) > 1:
            waits = list(si.on_wait)
            drain_inst.ins.sync_info = mybir.SyncInfo(
                on_wait=[waits[0]], on_update=list(si.on_update or [])
            )
            for w in waits[1:]:
                extra = self.nc.sync.drain()
                extra.ins.sync_info = mybir.SyncInfo(on_wait=[w], on_update=[])

        self.nc.all_engine_barrier()
        assert self.sems is not None
        popped = self.nc._tile_sem_poison_stack.pop()
        assert popped is self._sem_poison
        self.nc.clear_and_free_semaphores(
            list(self.sems.allocated().values())
        )
        self.nc.all_engine_barrier()

    tile.TileContext._drain_and_barrier = _drain_and_barrier
    tile.TileContext._tail_drain_patched = True


_patch_tail_drain()


def _split_multi_waits(nc):
    """This walrus build rejects instructions carrying more than one
    sync-wait (a DMA-HW-queue sem wait expands into several wait
    commands).  Give every instruction at most one wait; extras go onto
    same-engine NOPs inserted immediately before it."""

    def fresh_nop(engine):
        eng = nc.engines[engine]
        bi = eng.nop(nofuse=True)
        raw = bi.ins
        # nop() appended raw to the current bb -- remove it, we re-insert.
        for bb in nc.main_func.blocks:
            try:
                bb.instructions.remove(raw)
                break
            except ValueError:
                continue
        return raw

    for bb in nc.main_func.blocks:
        insts = bb.instructions
        i = 0
        while i < len(insts):
            ins = insts[i]
            si = getattr(ins, "sync_info", None)
            ow = list(si.on_wait) if (si is not None and si.on_wait) else []
            if len(ow) > 1:
                upd = list(si.on_update or [])
                for w in ow[:-1]:
                    nop = fresh_nop(ins.engine)
                    nop.sync_info = mybir.SyncInfo(on_wait=[w], on_update=[])
                    insts.insert(i, nop)
                    i += 1
                ins.sync_info = mybir.SyncInfo(on_wait=[ow[-1]],
                                               on_update=upd)
            i += 1


T, B, D = 100, 256, 1024
HH = [1024, 768, 512, 256, 10]  # H[l-1] -> H[l] for layer l in 1..4
NCORES = 8
BC = B // NCORES  # 32 batch per core
TB = 10           # max time block
# Time-block sizes: 10-step blocks early, 8-step blocks late (N=256 is
# the smallest moving dim that still streams at 1 cyc/row) so the
# pipeline-drain LIF chains at the end are 20% shorter.
TBS = [10] * 6 + [8] * 5
assert sum(TBS) == T and all(8 <= tb <= TB for tb in TBS)
TB0 = [sum(TBS[:i]) for i in range(len(TBS))]  # block start steps
NBLK = len(TBS)
RING = 2 * TB     # ring slots for cur/spike buffers
BETA = 0.95


def _kch(l):  # contraction chunks for layer l (input feature chunks)
    return (HH[l - 1] + 127) // 128


def _mch(l):  # output feature chunks
    return (HH[l] + 127) // 128


def _mpart(l):  # partitions used by last output chunk
    r = HH[l] % 128
    return 128 if r == 0 else r


def build_nc(repeat=1):
    nc = bass.Bass(target_bir_lowering=False, trn_type="TRN2")

    x_d = {
        p: nc.dram_tensor(f"x_{p}", [D, T * BC], F32R,
                          kind="ExternalInput") for p in PLANES
    }
    w_d = {}
    b_d = {}
    for l in range(1, 5):
        for p in PLANES:
            w_d[l, p] = nc.dram_tensor(
                f"w{l}{p}", [_kch(l), 128, HH[l]], F32R,
                kind="ExternalInput"
            )
        b_d[l] = nc.dram_tensor(f"b{l}", [HH[l]], F32, kind="ExternalInput")
    out_d = nc.dram_tensor("out", [10, BC], F32, kind="ExternalOutput")

    NB = TB * BC
    NBG = NBLK * repeat  # global block count

    with tile.TileContext(nc) as tc:
        from contextlib import ExitStack

        with ExitStack() as ctx:
            wpool = ctx.enter_context(tc.tile_pool(name="weights", bufs=1))
            xpool = ctx.enter_context(tc.tile_pool(name="xblk", bufs=2))
            spool = ctx.enter_context(tc.tile_pool(name="state", bufs=1))
            psum = ctx.enter_context(
                tc.tile_pool(name="psum", bufs=8, space="PSUM")
            )

            # ---- SBUF tiles for weights + biases (DMAs issued in the
            # startup-overlap order below, NOT here) ----
            w_sb = {}
            b_sb = {}
            for l in range(1, 5):
                kc = _kch(l)
                M = HH[l]
                for p in PLANES:
                    w_sb[l, p] = wpool.tile([128, kc * M], F32R,
                                            name=f"wsb{l}{p}")
                b_sb[l] = wpool.tile([128, _mch(l)], F32, name=f"bsb{l}")

            def dma_w_chunk(l, p, k):
                M = HH[l]
                nc.sync.dma_start(
                    w_sb[l, p][:, k * M:(k + 1) * M], w_d[l, p][k, :, :]
                )

            def dma_w(l):
                for p in PLANES:
                    nc.sync.dma_start(
                        w_sb[l, p].rearrange("q (k m) -> q k m", m=HH[l]),
                        w_d[l, p].rearrange("k q m -> q k m"),
                    )

            def dma_b(l):
                mp = _mpart(l)
                nc.sync.dma_start(
                    b_sb[l][:mp, :],
                    b_d[l].rearrange("(c q) -> q c", q=mp)
                    if _mch(l) > 1
                    else b_d[l][:].unsqueeze(-1),
                )

            # ---- persistent state ----
            m_t = {}    # membranes [128, Fl]  (l4: rows 0..9)
            tmp_t = {}
            s_t = {}    # spike rings, k-major: [128, kchunks * RING * BC]
            c_t = {}    # cur rings, t-major: [128, RING * Fl]
            for l in range(1, 5):
                mc = _mch(l)
                mp = _mpart(l)
                Fl = mc * BC
                pp = mp if mc == 1 else 128
                m_t[l] = spool.tile([pp, Fl], F32, name=f"mem{l}")
                tmp_t[l] = spool.tile([pp, Fl], F32, name=f"tmp{l}")
                s_t[l] = spool.tile([pp, mc * RING * BC], F32R,
                                    name=f"spk{l}")
                c_t[l] = spool.tile([pp, RING * Fl], F32, name=f"cur{l}")
                nc.vector.memset(m_t[l], 0.0)
                nc.vector.memset(s_t[l].bitcast(F32), 0.0)

            def lif_steps(l, bg):
                """Sequential LIF updates for layer l over global block
                bg.  All three ops on the DVE: same-engine in-order
                execution needs no semaphores on the serial chain (the
                fused compare+subtract form crashes the HW exec unit, so
                the spike ring mediates the reset as in the reference)."""
                mc = _mch(l)
                Fl = mc * BC
                sb = (bg % 2) * TB
                mem = m_t[l]
                tmp = tmp_t[l]
                mem3 = mem.rearrange("q (k b) -> q k b", b=BC)
                tmp3 = tmp.rearrange("q (k b) -> q k b", b=BC)
                s4 = s_t[l].rearrange("q (k r b) -> q k r b", r=RING, b=BC)
                for t in range(TB):
                    slot = sb + t
                    prev = (slot - 1) % RING
                    cur = c_t[l][:, slot * Fl:(slot + 1) * Fl]
                    # tmp = beta*mem + cur
                    nc.vector.scalar_tensor_tensor(
                        tmp, mem, BETA, cur, op0=ALU.mult, op1=ALU.add
                    )
                    # mem = tmp - s_prev
                    nc.vector.tensor_tensor(
                        mem3, tmp3, s4[:, :, prev, :], op=ALU.subtract
                    )
                    # s[slot] = mem > 1
                    nc.vector.tensor_scalar(
                        s4[:, :, slot, :], mem3, 1.0, None, op0=ALU.is_gt
                    )

            def drain_psum(l, ps, m, sb):
                mc = _mch(l)
                mp = _mpart(l)
                pp = mp if m == mc - 1 else 128
                c4 = c_t[l].rearrange("q (r k b) -> q r k b", r=RING, b=BC)
                nc.scalar.activation(
                    c4[:pp, sb:sb + TB, m, :],
                    ps.rearrange("q (t b) -> q t b", b=BC),
                    ACTF.Identity,
                    bias=b_sb[l][:pp, m:m + 1],
                )

            def layer1_matmul(bg, xb):
                """L1 matmuls, k-OUTER so block 0 can chase the DMA.  The
                per-psum-element accumulation sequence (k0:h@xh,h@xl,l@xh;
                k1:...) is identical to the m-outer form, so the numerics
                are unchanged."""
                M = HH[1]
                kc = _kch(1)
                mc = _mch(1)
                sb = (bg % 2) * TB
                ps = [psum.tile([128, NB], F32, name=f"ps1_{m}", tag="ps")
                      for m in range(mc)]
                for k in range(kc):
                    rhs_h = xb["h"][:, k * NB:(k + 1) * NB]
                    rhs_l = xb["l"][:, k * NB:(k + 1) * NB]
                    for m in range(mc):
                        wh = w_sb[1, "h"][:, k * M + m * 128:
                                          k * M + m * 128 + 128]
                        wl = w_sb[1, "l"][:, k * M + m * 128:
                                          k * M + m * 128 + 128]
                        nc.tensor.matmul(ps[m], wh, rhs_h,
                                         start=(k == 0), stop=False)
                        nc.tensor.matmul(ps[m], wh, rhs_l,
                                         start=False, stop=False)
                        nc.tensor.matmul(ps[m], wl, rhs_h,
                                         start=False, stop=(k == kc - 1))
                for m in range(mc):
                    drain_psum(1, ps[m], m, sb)

            def layer_matmul(l, bg):
                """Spike-layer matmuls (m-outer, k/plane-inner)."""
                mc = _mch(l)
                kc = _kch(l)
                M = HH[l]
                mp = _mpart(l)
                sb = (bg % 2) * TB
                sl = s_t[l - 1]
                for m in range(mc):
                    pp = mp if m == mc - 1 else 128
                    ps = psum.tile([pp, NB], F32, name=f"ps{l}", tag="ps")
                    n = 2 * kc
                    i = 0
                    for k in range(kc):
                        rhs = sl[:, k * RING * BC + sb * BC:
                                 k * RING * BC + sb * BC + NB]
                        for p in PLANES:
                            lhsT = w_sb[l, p][:, k * M + m * 128:
                                              k * M + m * 128 + pp]
                            nc.tensor.matmul(
                                ps, lhsT, rhs,
                                start=(i == 0), stop=(i == n - 1),
                            )
                            i += 1
                    drain_psum(l, ps, m, sb)

            def dma_x(bg, split=False):
                blk = bg % NBLK
                tiles = {}
                for p in PLANES:
                    tiles[p] = xpool.tile([128, _kch(1) * NB], F32R,
                                          name=f"xb{p}", tag=f"xb{p}")
                if split:
                    # per-k pieces, interleaved with the matching w1
                    # chunks, so L1 block 0 starts after the first chunk
                    # group lands (~4us) and chases the DMA stream
                    for k in range(_kch(1)):
                        for p in PLANES:
                            nc.sync.dma_start(
                                tiles[p][:, k * NB:(k + 1) * NB],
                                x_d[p][k * 128:(k + 1) * 128,
                                       blk * NB:(blk + 1) * NB],
                            )
                        for p in PLANES:
                            dma_w_chunk(1, p, k)
                else:
                    for p in PLANES:
                        nc.sync.dma_start(
                            tiles[p].rearrange("q (k n) -> q k n", n=NB),
                            x_d[p][:, blk * NB:(blk + 1) * NB].rearrange(
                                "(k q) n -> q k n", q=128
                            ),
                        )
                return tiles

            # ---- DMA prologue, ordered for startup overlap: x block 0
            # and w1 interleaved per-k chunk (PE starts ~4us in), then
            # x1, then the remaining layers' weights in consumption
            # order.  All on the in-order SP queue. ----
            x_tiles = {}
            x_tiles[0] = dma_x(0, split=True)
            dma_b(1)
            x_tiles[1] = dma_x(1)
            dma_w(2)
            dma_b(2)
            dma_w(3)
            dma_b(3)
            dma_w(4)
            dma_b(4)

            # Software pipeline: at tick t, layer l's matmuls cover block
            # t-(l-1); LIF chains for the same blocks are emitted in the
            # SAME tick after all matmuls (ascending data-ready order
            # lif1..lif4 is by construction: lif_l gates on its cur drain).
            for tick in range(NBG + 3):
                if tick + 2 < NBG:
                    x_tiles[tick + 2] = dma_x(tick + 2)
                for l in (1, 2, 3, 4):
                    b = tick - (l - 1)
                    if not (0 <= b < NBG):
                        continue
                    if l == 1:
                        layer1_matmul(b, x_tiles[b])
                    else:
                        layer_matmul(l, b)
                for l in (1, 2, 3, 4):
                    b = tick - (l - 1)
                    if 0 <= b < NBG:
                        lif_steps(l, b)

            nc.sync.dma_start(out_d[:, :], m_t[4])

    _split_multi_waits(nc)
    return nc


_NC_CACHE = None


def _get_nc():
    global _NC_CACHE
    if _NC_CACHE is None:
        _NC_CACHE = build_nc()
    return _NC_CACHE


def _rne11(a):
    """Round fp32 mantissa to 11 bits (RNE) -- the f32r operand grid."""
    u = np.ascontiguousarray(a, np.float32).view(np.uint32).astype(np.uint64)
    zb = 12  # 23 - 11
    lsb = (u >> zb) & 1
    add = lsb + ((1 << (zb - 1)) - 1)
    r = ((u + add) >> zb) << zb
    return r.astype(np.uint32).view(np.float32)


def _split2_11(a):
    """fp32 -> two 11-bit-mantissa planes with h + l == a exactly."""
    a = np.asarray(a, np.float32)
    h = _rne11(a)
    l = (a - h).astype(np.float32)
    return h, l


def prep_inputs(x, W1, b1, W2, b2, W3, b3, W4, b4):
    """Full inputs -> per-core in_maps."""
    Ws = {1: W1, 2: W2, 3: W3, 4: W4}
    bs = {1: b1, 2: b2, 3: b3, 4: b4}
    shared = {}
    for l in range(1, 5):
        wt = np.ascontiguousarray(
            np.asarray(Ws[l], np.float32).T.reshape(_kch(l), 128, HH[l])
        )
        wh, wl = _split2_11(wt)
        shared[f"w{l}h"] = wh
        shared[f"w{l}l"] = wl
        shared[f"b{l}"] = np.ascontiguousarray(bs[l], dtype=np.float32)
    in_maps = []
    for c in range(NCORES):
        xc = np.asarray(x[:, c * BC:(c + 1) * BC, :], np.float32)
        xc = np.ascontiguousarray(xc.transpose(2, 0, 1).reshape(D, T * BC))
        xh, xl = _split2_11(xc)
        m = {"x_h": xh, "x_l": xl}
        m.update(shared)
        in_maps.append(m)
    return in_maps


def run(in_maps, trace=False):
    nc = _get_nc()
    return bass_utils.run_bass_kernel_spmd(
        nc, in_maps, core_ids=list(range(NCORES)), trace=trace
    )


def kernel(**inputs):
    in_maps = prep_inputs(**inputs)
    res = run(in_maps)
    out = np.empty((B, 10), dtype=np.float32)
    for c in range(NCORES):
        out[c * BC:(c + 1) * BC, :] = res.results[c]["out"].T
    return out


def bench(in_maps, iters=20, nc=None):
    """Repeat-execute the kernel via a cached sharded jit; returns list of
    per-call wall times (seconds).  Mirrors bass2jax.run_bass_via_pjrt's
    multi-core path but keeps inputs device-resident across calls."""
    import time

    import jax
    import concourse.mybir as mybir_
    from jax.sharding import Mesh, PartitionSpec
    from jax.experimental.shard_map import shard_map
    from concourse import bass2jax

    bass2jax.install_neuronx_cc_hook()
    if nc is None:
        nc = _get_nc()

    part_name = (nc.partition_id_tensor.name
                 if nc.partition_id_tensor else None)
    in_names, out_names, out_avals, zero_outs = [], [], [], []
    for alloc in nc.m.functions[0].allocations:
        if not isinstance(alloc, mybir_.MemoryLocationSet):
            continue
        name = alloc.memorylocations[0].name
        if alloc.kind == "ExternalInput":
            if name != part_name:
                in_names.append(name)
        elif alloc.kind == "ExternalOutput":
            out_names.append(name)
            shape = tuple(alloc.tensor_shape)
            dtype = mybir_.dt.np(alloc.dtype)
            out_avals.append(jax.core.ShapedArray(shape, dtype))
            zero_outs.append(np.zeros(shape, dtype))
    n_params = len(in_names)
    all_in_names = in_names + out_names
    if part_name is not None:
        all_in_names = all_in_names + [part_name]

    def _body(*args):
        operands = list(args)
        if part_name is not None:
            operands.append(bass2jax.partition_id_tensor())
        outs = bass2jax._bass_exec_p.bind(
            *operands,
            out_avals=tuple(out_avals),
            in_names=tuple(all_in_names),
            out_names=tuple(out_names),
            lowering_input_output_aliases=(),
            sim_require_finite=True,
            sim_require_nnan=True,
            nc=nc,
        )
        return tuple(outs)

    devices = jax.devices()[:NCORES]
    mesh = Mesh(np.asarray(devices), ("core",))
    n_outs = len(out_names)
    sharded = jax.jit(
        shard_map(
            _body, mesh=mesh,
            in_specs=(PartitionSpec("core"),) * (n_params + n_outs),
            out_specs=(PartitionSpec("core"),) * n_outs,
            check_rep=False,
        ),
        donate_argnums=tuple(range(n_params, n_params + n_outs)),
        keep_unused=True,
    )
    concat_in = [
        np.concatenate([np.asarray(m[nm]) for m in in_maps], axis=0)
        for nm in in_names
    ]
    concat_in = jax.device_put(concat_in)
    zeros = [
        np.zeros((NCORES * z.shape[0], *z.shape[1:]), z.dtype)
        for z in zero_outs
    ]
    # warmup (compile)
    out = sharded(*concat_in, *zeros)
    jax.block_until_ready(out)
    times = []
    for _ in range(iters):
        t0 = time.perf_counter()
        out = sharded(*concat_in, *zeros)
        jax.block_until_ready(out)
        times.append(time.perf_counter() - t0)
    return times



# revision 38
# speedup vs baseline: 1.0238x; 1.0238x over previous
"""Trainium2 Bass kernel for EnhancedSpikingAudioNet (4-layer LIF SNN).

Network (eval mode): for t in 0..99:
    s1,m1 = LIF(x_t @ W1.T + b1, m1)
    s2,m2 = LIF(s1 @ W2.T + b2, m2)
    s3,m3 = LIF(s2 @ W3.T + b3, m3)
    s4,m4 = LIF(s3 @ W4.T + b4, m4)
returns m4 (final step), shape [B=256, 10].

LIF (snnTorch Leaky, reset_mechanism='subtract', beta=.95, thr=1):
    reset = (m_prev > 1);  m = beta*m_prev + cur - reset;  s = (m > 1)
Note reset(t) == spike(t-1), so only s needs materializing.

Strategy: data-parallel over batch (32 per core, 8 cores).  Inside a
core, time is blocked (TB=10): all matmuls for a block are batched over
the block's 10 steps (moving free dim N=320); only the per-step LIF
update is sequential (3 DVE ops per layer per step, all on one engine
so the serial chain carries no cross-engine semaphores).  Layout:
features on partitions (128-chunks), (t, batch) on the free dim.  PSUM
is drained to SBUF by ScalarE with the layer bias fused in.  Startup:
x block 0 and w1 stream in per-k-chunk interleaved so the PE starts
~3us in (L1 is k-outer to chase the DMA); remaining weights follow in
consumption order.

Numerics: the spike cascade amplifies matmul noise (a plain f32r
matmul gives ~16% output error; even exact-fp32 summation-order noise
gives ~1.6%), so matmuls must be fp32-faithful.  Hardware probing
established: float32r = operands rounded RNE to 11 mantissa bits
(FP22 e10m11), then EXACT products with clean fp32 accumulation (a
pre-rounded-operand probe matches an exact model to 1e-7), at full PE
rate (1 cyc/row) for moving dims >= 256.  Therefore every fp32 tensor
is split host-side into two 11-bit planes (h = rne11(a), l = a-h; the
residual fits 12 significand bits, so a == h+l exactly and both planes
survive the hw operand rounding unchanged).  Weights use 2 planes;
spikes are 0/1 (f32r-exact, single copy); x uses 2 planes.  Layer 1
accumulates wh@xh + wh@xl + wl@xh (dropping wl@xl ~ 2^-24), layers 2-4
accumulate wh@s + wl@s.  Everything streams at 1 cycle/row and the
result is fp32-exact (hw-validated: bit-identical to the jax CPU
reference).
"""

import os
import sys

import numpy as np

for _p in ("/opt/trn_rl_repo",):
    if os.path.isdir(_p) and _p not in sys.path:
        sys.path.insert(0, _p)

import concourse.bass as bass
import concourse.mybir as mybir
import concourse.tile as tile
from concourse import bass_utils

F32 = mybir.dt.float32
F32R = mybir.dt.float32r
ALU = mybir.AluOpType
ACTF = mybir.ActivationFunctionType
PLANES = ("h", "l")  # 11-bit f32r planes


def _patch_tail_drain():
    """This container's walrus allows only ONE sync-wait on a Drain
    instruction; Tile's kernel-tail drain can carry several (one per DMA
    HW queue).  Spread the waits across consecutive drains instead."""
    from concourse.vector_clock import ScopedClock

    if getattr(tile.TileContext, "_tail_drain_patched", False):
        return

    def _drain_and_barrier(self, tick_clock, wait_clock):
        drain_inst = self.nc.sync.drain()
        wait_clock.add_sem_waits(
            drain_inst.ins, ScopedClock({None: tick_clock.global_clock})
        )
        si = drain_inst.ins.sync_info
        if si is not None and si.on_wait and len(si.on_wait) > 1:
            waits = list(si.on_wait)
            drain_inst.ins.sync_info = mybir.SyncInfo(
                on_wait=[waits[0]], on_update=list(si.on_update or [])
            )
            for w in waits[1:]:
                extra = self.nc.sync.drain()
                extra.ins.sync_info = mybir.SyncInfo(on_wait=[w], on_update=[])

        self.nc.all_engine_barrier()
        assert self.sems is not None
        popped = self.nc._tile_sem_poison_stack.pop()
        assert popped is self._sem_poison
        self.nc.clear_and_free_semaphores(
            list(self.sems.allocated().values())
        )
        self.nc.all_engine_barrier()

    tile.TileContext._drain_and_barrier = _drain_and_barrier
    tile.TileContext._tail_drain_patched = True


_patch_tail_drain()


def _split_multi_waits(nc):
    """This walrus build rejects instructions carrying more than one
    sync-wait (a DMA-HW-queue sem wait expands into several wait
    commands).  Give every instruction at most one wait; extras go onto
    same-engine NOPs inserted immediately before it."""

    def fresh_nop(engine):
        eng = nc.engines[engine]
        bi = eng.nop(nofuse=True)
        raw = bi.ins
        # nop() appended raw to the current bb -- remove it, we re-insert.
        for bb in nc.main_func.blocks:
            try:
                bb.instructions.remove(raw)
                break
            except ValueError:
                continue
        return raw

    for bb in nc.main_func.blocks:
        insts = bb.instructions
        i = 0
        while i < len(insts):
            ins = insts[i]
            si = getattr(ins, "sync_info", None)
            ow = list(si.on_wait) if (si is not None and si.on_wait) else []
            if len(ow) > 1:
                upd = list(si.on_update or [])
                for w in ow[:-1]:
                    nop = fresh_nop(ins.engine)
                    nop.sync_info = mybir.SyncInfo(on_wait=[w], on_update=[])
                    insts.insert(i, nop)
                    i += 1
                ins.sync_info = mybir.SyncInfo(on_wait=[ow[-1]],
                                               on_update=upd)
            i += 1


T, B, D = 100, 256, 1024
HH = [1024, 768, 512, 256, 10]  # H[l-1] -> H[l] for layer l in 1..4
NCORES = 8
BC = B // NCORES  # 32 batch per core
TB = 10           # max time block
# Time-block sizes: 10-step blocks early, 8-step blocks late (N=256 is
# the smallest moving dim that still streams at 1 cyc/row) so the
# pipeline-drain LIF chains at the end are 20% shorter.
TBS = [10] * 6 + [8] * 5
assert sum(TBS) == T and all(8 <= tb <= TB for tb in TBS)
TB0 = [sum(TBS[:i]) for i in range(len(TBS))]  # block start steps
NBLK = len(TBS)
RING = 2 * TB     # ring slots for cur/spike buffers
BETA = 0.95
DEVICE_SPLIT = False  # bisect: host planes


def _kch(l):  # contraction chunks for layer l (input feature chunks)
    return (HH[l - 1] + 127) // 128


def _mch(l):  # output feature chunks
    return (HH[l] + 127) // 128


def _mpart(l):  # partitions used by last output chunk
    r = HH[l] % 128
    return 128 if r == 0 else r


def build_nc(repeat=1):
    nc = bass.Bass(target_bir_lowering=False, trn_type="TRN2")

    if DEVICE_SPLIT:
        x_d = nc.dram_tensor("x_f", [D, T * BC], F32,
                             kind="ExternalInput")
    else:
        x_dp = {
            p: nc.dram_tensor(f"x_{p}", [D, T * BC], F32R,
                              kind="ExternalInput") for p in PLANES
        }
    w_d = {}
    b_d = {}
    for l in range(1, 5):
        for p in PLANES:
            w_d[l, p] = nc.dram_tensor(
                f"w{l}{p}", [_kch(l), 128, HH[l]], F32R,
                kind="ExternalInput"
            )
        b_d[l] = nc.dram_tensor(f"b{l}", [HH[l]], F32, kind="ExternalInput")
    out_d = nc.dram_tensor("out", [10, BC], F32, kind="ExternalOutput")

    NB = TB * BC
    NBG = NBLK * repeat  # global block count

    with tile.TileContext(nc) as tc:
        from contextlib import ExitStack

        with ExitStack() as ctx:
            wpool = ctx.enter_context(tc.tile_pool(name="weights", bufs=1))
            xpool = ctx.enter_context(tc.tile_pool(name="xblk", bufs=2))
            xrpool = ctx.enter_context(tc.tile_pool(name="xraw", bufs=1))
            spool = ctx.enter_context(tc.tile_pool(name="state", bufs=1))
            psum = ctx.enter_context(
                tc.tile_pool(name="psum", bufs=8, space="PSUM")
            )

            # ---- SBUF tiles for weights + biases (DMAs issued in the
            # startup-overlap order below, NOT here) ----
            w_sb = {}
            b_sb = {}
            for l in range(1, 5):
                kc = _kch(l)
                M = HH[l]
                for p in PLANES:
                    w_sb[l, p] = wpool.tile([128, kc * M], F32R,
                                            name=f"wsb{l}{p}")
                b_sb[l] = wpool.tile([128, _mch(l)], F32, name=f"bsb{l}")

            def dma_w_chunk(l, p, k):
                M = HH[l]
                nc.sync.dma_start(
                    w_sb[l, p][:, k * M:(k + 1) * M], w_d[l, p][k, :, :]
                )

            def dma_w(l):
                for p in PLANES:
                    nc.sync.dma_start(
                        w_sb[l, p].rearrange("q (k m) -> q k m", m=HH[l]),
                        w_d[l, p].rearrange("k q m -> q k m"),
                    )

            def dma_b(l):
                mp = _mpart(l)
                nc.sync.dma_start(
                    b_sb[l][:mp, :],
                    b_d[l].rearrange("(c q) -> q c", q=mp)
                    if _mch(l) > 1
                    else b_d[l][:].unsqueeze(-1),
                )

            # ---- persistent state ----
            m_t = {}    # membranes [128, Fl]  (l4: rows 0..9)
            tmp_t = {}
            s_t = {}    # spike rings, k-major: [128, kchunks * RING * BC]
            c_t = {}    # cur rings, t-major: [128, RING * Fl]
            for l in range(1, 5):
                mc = _mch(l)
                mp = _mpart(l)
                Fl = mc * BC
                pp = mp if mc == 1 else 128
                m_t[l] = spool.tile([pp, Fl], F32, name=f"mem{l}")
                tmp_t[l] = spool.tile([pp, Fl], F32, name=f"tmp{l}")
                if l == 4:
                    tmp2_4 = spool.tile([pp, Fl], F32, name="tmp2_4")
                s_t[l] = spool.tile([pp, mc * RING * BC], F32R,
                                    name=f"spk{l}")
                c_t[l] = spool.tile([pp, RING * Fl], F32, name=f"cur{l}")
                nc.vector.memset(m_t[l], 0.0)
                nc.vector.memset(s_t[l].bitcast(F32), 0.0)

            def lif_steps(l, bg):
                """Sequential LIF updates for layer l over global block
                bg.  All three ops on the DVE: same-engine in-order
                execution needs no semaphores on the serial chain (the
                fused compare+subtract form crashes the HW exec unit, so
                the spike ring mediates the reset as in the reference)."""
                mc = _mch(l)
                Fl = mc * BC
                sb = (bg % 2) * TB
                mem = m_t[l]
                tmp = tmp_t[l]
                mem3 = mem.rearrange("q (k b) -> q k b", b=BC)
                tmp3 = tmp.rearrange("q (k b) -> q k b", b=BC)
                s4 = s_t[l].rearrange("q (k r b) -> q k r b", r=RING, b=BC)
                for t in range(TBS[bg % NBLK]):
                    slot = sb + t
                    if t > 0:
                        prev = slot - 1
                    elif bg == 0:
                        prev = RING - 1  # zeroed ring: no spike before t=0
                    else:
                        # last written slot of the previous (possibly
                        # shorter) block's ring half
                        prev = ((bg - 1) % 2) * TB \
                            + TBS[(bg - 1) % NBLK] - 1
                    cur = c_t[l][:, slot * Fl:(slot + 1) * Fl]
                    if l == 4 and bg < NBG - 2:
                        # layer 4 runs on Pool (unfused; Pool's fused stt
                        # doesn't compile) to take its serial chains off
                        # the DVE, which binds the pipeline drain.  The
                        # final block stays on the DVE (3-op fused chain
                        # is shorter and ends the kernel).
                        # Per-stage rounding is HW-verified identical.
                        nc.gpsimd.tensor_scalar(
                            tmp, mem, BETA, None, op0=ALU.mult
                        )
                        nc.gpsimd.tensor_tensor(
                            tmp2_4, tmp, cur, op=ALU.add
                        )
                        nc.gpsimd.tensor_tensor(
                            mem3,
                            tmp2_4.rearrange("q (k b) -> q k b", b=BC),
                            s4[:, :, prev, :], op=ALU.subtract
                        )
                        nc.gpsimd.tensor_scalar(
                            s4[:, :, slot, :], mem3, 1.0, None,
                            op0=ALU.is_gt
                        )
                        continue
                    # tmp = beta*mem + cur
                    nc.vector.scalar_tensor_tensor(
                        tmp, mem, BETA, cur, op0=ALU.mult, op1=ALU.add
                    )
                    # mem = tmp - s_prev
                    nc.vector.tensor_tensor(
                        mem3, tmp3, s4[:, :, prev, :], op=ALU.subtract
                    )
                    # s[slot] = mem > 1
                    nc.vector.tensor_scalar(
                        s4[:, :, slot, :], mem3, 1.0, None, op0=ALU.is_gt
                    )

            def drain_psum(l, ps, m, sb, tb, on_dve=False):
                mc = _mch(l)
                mp = _mpart(l)
                pp = mp if m == mc - 1 else 128
                c4 = c_t[l].rearrange("q (r k b) -> q r k b", r=RING, b=BC)
                dst = c4[:pp, sb:sb + tb, m, :]
                src = ps[:, :tb * BC].rearrange("q (t b) -> q t b", b=BC)
                if on_dve:
                    # same single fp32 add as ACT Identity-with-bias
                    nc.vector.tensor_scalar(
                        dst, src, b_sb[l][:pp, m:m + 1], None, op0=ALU.add
                    )
                else:
                    nc.scalar.activation(
                        dst, src, ACTF.Identity,
                        bias=b_sb[l][:pp, m:m + 1],
                    )

            def layer1_matmul(bg, xb):
                """L1 matmuls, k-OUTER so block 0 can chase the DMA.  The
                per-psum-element accumulation sequence (k0:h@xh,h@xl,l@xh;
                k1:...) is identical to the m-outer form, so the numerics
                are unchanged."""
                M = HH[1]
                kc = _kch(1)
                mc = _mch(1)
                tb = TBS[bg % NBLK]
                nb = tb * BC
                sb = (bg % 2) * TB
                last = (bg == NBG - 1)
                ps = [psum.tile([128, NB], F32, name=f"ps1_{m}", tag="ps")
                      for m in range(mc)]
                for k in range(kc):
                    rhs_h = xb["h"][:, k * NB:k * NB + nb]
                    rhs_l = xb["l"][:, k * NB:k * NB + nb]
                    for m in range(mc):
                        wh = w_sb[1, "h"][:, k * M + m * 128:
                                          k * M + m * 128 + 128]
                        wl = w_sb[1, "l"][:, k * M + m * 128:
                                          k * M + m * 128 + 128]
                        nc.tensor.matmul(ps[m][:, :nb], wh, rhs_h,
                                         start=(k == 0), stop=False)
                        nc.tensor.matmul(ps[m][:, :nb], wh, rhs_l,
                                         start=False, stop=False)
                        nc.tensor.matmul(ps[m][:, :nb], wl, rhs_h,
                                         start=False, stop=(k == kc - 1))
                for m in range(mc):
                    # final block: split drains ACT/DVE to halve the
                    # serialization ahead of the last LIF chain
                    drain_psum(1, ps[m], m, sb, tb,
                               on_dve=(last and m >= mc - mc // 2))

            def layer_matmul(l, bg):
                """Spike-layer matmuls (m-outer, k/plane-inner)."""
                mc = _mch(l)
                kc = _kch(l)
                M = HH[l]
                mp = _mpart(l)
                tb = TBS[bg % NBLK]
                nb = tb * BC
                sb = (bg % 2) * TB
                last = (bg == NBG - 1)
                sl = s_t[l - 1]
                for m in range(mc):
                    pp = mp if m == mc - 1 else 128
                    ps = psum.tile([pp, NB], F32, name=f"ps{l}", tag="ps")
                    n = 2 * kc
                    i = 0
                    for k in range(kc):
                        rhs = sl[:, k * RING * BC + sb * BC:
                                 k * RING * BC + sb * BC + nb]
                        for p in PLANES:
                            lhsT = w_sb[l, p][:, k * M + m * 128:
                                              k * M + m * 128 + pp]
                            nc.tensor.matmul(
                                ps[:, :nb], lhsT, rhs,
                                start=(i == 0), stop=(i == n - 1),
                            )
                            i += 1
                    drain_psum(l, ps, m, sb, tb,
                               on_dve=(last and m >= mc - mc // 2))

            def dma_x(bg, chunked=False):
                """DMA the raw fp32 x block; split_x materializes the
                f32r planes."""
                blk = bg % NBLK
                nb = TBS[blk] * BC
                c0 = TB0[blk] * BC
                xh = xpool.tile([128, _kch(1) * NB], F32R,
                                name="xbh", tag="xbh")
                xl = xpool.tile([128, _kch(1) * NB], F32R,
                                name="xbl", tag="xbl")
                if not DEVICE_SPLIT:
                    tiles = {"h": xh, "l": xl}
                    if chunked:
                        for k in range(_kch(1)):
                            for p in PLANES:
                                nc.sync.dma_start(
                                    tiles[p][:, k * NB:k * NB + nb],
                                    x_dp[p][k * 128:(k + 1) * 128,
                                            c0:c0 + nb],
                                )
                            if bg == 0:
                                for p in PLANES:
                                    dma_w_chunk(1, p, k)
                    else:
                        for p in PLANES:
                            nc.sync.dma_start(
                                tiles[p].rearrange(
                                    "q (k n) -> q k n", n=NB)[:, :, :nb],
                                x_dp[p][:, c0:c0 + nb].rearrange(
                                    "(k q) n -> q k n", q=128
                                ),
                            )
                    return tiles
                xr = xrpool.tile([128, _kch(1) * NB], F32,
                                 name="xbr", tag="xbr")
                if chunked:
                    # per-k pieces (for block 0 interleaved with w1) so
                    # L1 blocks 0/1 can chase the DMA stream
                    for k in range(_kch(1)):
                        nc.sync.dma_start(
                            xr[:, k * NB:k * NB + nb],
                            x_d[k * 128:(k + 1) * 128, c0:c0 + nb],
                        )
                        if bg == 0:
                            for p in PLANES:
                                dma_w_chunk(1, p, k)
                else:
                    nc.sync.dma_start(
                        xr.rearrange(
                            "q (k n) -> q k n", n=NB)[:, :, :nb],
                        x_d[:, c0:c0 + nb].rearrange(
                            "(k q) n -> q k n", q=128
                        ),
                    )
                return {"r": xr, "h": xh, "l": xl}

            def split_x(bg, xb):
                """f32r plane split on Pool (it is otherwise idle): an
                f32r-typed ALU output rounds RNE to 11 mantissa bits
                (HW-verified == rne11 bitwise), so h = round(x) in one op
                and l = x - h is exact with both planes on the f32r grid
                (h + l == x)."""
                if not DEVICE_SPLIT:
                    return
                nb = TBS[bg % NBLK] * BC
                xr = xb["r"]
                for k in range(_kch(1)):
                    xk = xr[:, k * NB:k * NB + nb]
                    hk = xb["h"][:, k * NB:k * NB + nb]
                    nc.gpsimd.tensor_scalar(hk, xk, 1.0, None,
                                            op0=ALU.mult)
                    nc.gpsimd.tensor_tensor(
                        xb["l"][:, k * NB:k * NB + nb],
                        xk, hk.bitcast(F32), op=ALU.subtract
                    )

            # ---- DMA prologue, ordered for startup overlap: x block 0
            # and w1 interleaved per-k chunk (PE starts ~4us in), then
            # x1, then the remaining layers' weights in consumption
            # order.  All on the in-order SP queue.  The Pool split ops
            # chase the per-k DMAs for blocks 0 and 1. ----
            x_tiles = {}
            x_tiles[0] = dma_x(0, chunked=True)
            split_x(0, x_tiles[0])
            dma_b(1)
            x_tiles[1] = dma_x(1, chunked=True)
            split_x(1, x_tiles[1])
            dma_w(2)
            dma_b(2)
            dma_w(3)
            dma_b(3)
            dma_w(4)
            dma_b(4)

            # Software pipeline: at tick t, layer l's matmuls cover block
            # t-(l-1); LIF chains for the same blocks are emitted in the
            # SAME tick after all matmuls (ascending data-ready order
            # lif1..lif4 is by construction: lif_l gates on its cur drain).
            # Cross-engine waits are per-engine counting semaphores: a PE
            # instruction's threshold counts EVERY earlier-emitted DVE
            # update, so a lif chain emitted between a producer chain and
            # its consuming matmul inflates the wait.  In the drain
            # region (ticks >= NBG-1, where the PE is chain-bound) emit
            # only the newest block's lif before the next tick's matmuls
            # and defer the other lif chains until after them.
            pending = set()
            for tick in range(NBG + 3):
                if tick + 2 < NBG:
                    x_tiles[tick + 2] = dma_x(tick + 2)
                    split_x(tick + 2, x_tiles[tick + 2])
                for l in (1, 2, 3, 4):
                    b = tick - (l - 1)
                    if not (0 <= b < NBG):
                        continue
                    if (l - 1, b) in pending:
                        # deferred producer chain goes right before its
                        # consuming matmul so the sem threshold stays tight
                        lif_steps(l - 1, b)
                        pending.discard((l - 1, b))
                    if l == 1:
                        layer1_matmul(b, x_tiles[b])
                    else:
                        layer_matmul(l, b)
                newest_done = False
                for l in (1, 2, 3, 4):
                    b = tick - (l - 1)
                    if not (0 <= b < NBG):
                        continue
                    # lif4 runs on Pool (except the final block) and so
                    # never inflates a PE sem threshold; don't defer it.
                    if (tick >= NBG - 1 and newest_done
                            and not (l == 4 and b < NBG - 1)):
                        pending.add((l, b))
                    else:
                        lif_steps(l, b)
                        newest_done = True
            assert not pending

            nc.sync.dma_start(out_d[:, :], m_t[4])

    _split_multi_waits(nc)
    return nc


_NC_CACHE = None


def _get_nc():
    global _NC_CACHE
    if _NC_CACHE is None:
        _NC_CACHE = build_nc()
    return _NC_CACHE


def _rne11(a):
    """Round fp32 mantissa to 11 bits (RNE) -- the f32r operand grid."""
    u = np.ascontiguousarray(a, np.float32).view(np.uint32).astype(np.uint64)
    zb = 12  # 23 - 11
    lsb = (u >> zb) & 1
    add = lsb + ((1 << (zb - 1)) - 1)
    r = ((u + add) >> zb) << zb
    return r.astype(np.uint32).view(np.float32)


def _split2_11(a):
    """fp32 -> two 11-bit-mantissa planes with h + l == a exactly."""
    a = np.asarray(a, np.float32)
    h = _rne11(a)
    l = (a - h).astype(np.float32)
    return h, l


def prep_inputs(x, W1, b1, W2, b2, W3, b3, W4, b4):
    """Full inputs -> per-core in_maps."""
    Ws = {1: W1, 2: W2, 3: W3, 4: W4}
    bs = {1: b1, 2: b2, 3: b3, 4: b4}
    shared = {}
    for l in range(1, 5):
        wt = np.ascontiguousarray(
            np.asarray(Ws[l], np.float32).T.reshape(_kch(l), 128, HH[l])
        )
        wh, wl = _split2_11(wt)
        shared[f"w{l}h"] = wh
        shared[f"w{l}l"] = wl
        shared[f"b{l}"] = np.ascontiguousarray(bs[l], dtype=np.float32)
    in_maps = []
    for c in range(NCORES):
        xc = np.asarray(x[:, c * BC:(c + 1) * BC, :], np.float32)
        xc = np.ascontiguousarray(xc.transpose(2, 0, 1).reshape(D, T * BC))
        if DEVICE_SPLIT:
            m = {"x_f": xc}
        else:
            xh, xl = _split2_11(xc)
            m = {"x_h": xh, "x_l": xl}
        m.update(shared)
        in_maps.append(m)
    return in_maps


def run(in_maps, trace=False):
    nc = _get_nc()
    return bass_utils.run_bass_kernel_spmd(
        nc, in_maps, core_ids=list(range(NCORES)), trace=trace
    )


def kernel(**inputs):
    in_maps = prep_inputs(**inputs)
    res = run(in_maps)
    out = np.empty((B, 10), dtype=np.float32)
    for c in range(NCORES):
        out[c * BC:(c + 1) * BC, :] = res.results[c]["out"].T
    return out


def bench(in_maps, iters=20, nc=None):
    """Repeat-execute the kernel via a cached sharded jit; returns list of
    per-call wall times (seconds).  Mirrors bass2jax.run_bass_via_pjrt's
    multi-core path but keeps inputs device-resident across calls."""
    import time

    import jax
    import concourse.mybir as mybir_
    from jax.sharding import Mesh, PartitionSpec
    from jax.experimental.shard_map import shard_map
    from concourse import bass2jax

    bass2jax.install_neuronx_cc_hook()
    if nc is None:
        nc = _get_nc()

    part_name = (nc.partition_id_tensor.name
                 if nc.partition_id_tensor else None)
    in_names, out_names, out_avals, zero_outs = [], [], [], []
    for alloc in nc.m.functions[0].allocations:
        if not isinstance(alloc, mybir_.MemoryLocationSet):
            continue
        name = alloc.memorylocations[0].name
        if alloc.kind == "ExternalInput":
            if name != part_name:
                in_names.append(name)
        elif alloc.kind == "ExternalOutput":
            out_names.append(name)
            shape = tuple(alloc.tensor_shape)
            dtype = mybir_.dt.np(alloc.dtype)
            out_avals.append(jax.core.ShapedArray(shape, dtype))
            zero_outs.append(np.zeros(shape, dtype))
    n_params = len(in_names)
    all_in_names = in_names + out_names
    if part_name is not None:
        all_in_names = all_in_names + [part_name]

    def _body(*args):
        operands = list(args)
        if part_name is not None:
            operands.append(bass2jax.partition_id_tensor())
        outs = bass2jax._bass_exec_p.bind(
            *operands,
            out_avals=tuple(out_avals),
            in_names=tuple(all_in_names),
            out_names=tuple(out_names),
            lowering_input_output_aliases=(),
            sim_require_finite=True,
            sim_require_nnan=True,
            nc=nc,
        )
        return tuple(outs)

    devices = jax.devices()[:NCORES]
    mesh = Mesh(np.asarray(devices), ("core",))
    n_outs = len(out_names)
    sharded = jax.jit(
        shard_map(
            _body, mesh=mesh,
            in_specs=(PartitionSpec("core"),) * (n_params + n_outs),
            out_specs=(PartitionSpec("core"),) * n_outs,
            check_rep=False,
        ),
        donate_argnums=tuple(range(n_params, n_params + n_outs)),
        keep_unused=True,
    )
    concat_in = [
        np.concatenate([np.asarray(m[nm]) for m in in_maps], axis=0)
        for nm in in_names
    ]
    concat_in = jax.device_put(concat_in)
    zeros = [
        np.zeros((NCORES * z.shape[0], *z.shape[1:]), z.dtype)
        for z in zero_outs
    ]
    # warmup (compile)
    out = sharded(*concat_in, *zeros)
    jax.block_until_ready(out)
    times = []
    for _ in range(iters):
        t0 = time.perf_counter()
        out = sharded(*concat_in, *zeros)
        jax.block_until_ready(out)
        times.append(time.perf_counter() - t0)
    return times

